# revision 1
# baseline (speedup 1.0000x reference)
"""Trainium2 Bass kernel for nn_Conv2dModulated (modulated transposed conv + blur).

Math restructure (validated vs reference to 5e-7 rel in fp32):
  s = w @ affine_w.T + affine_b + 1                    (B, CIN)  host
  d = rsqrt(s^2 @ sum_kk(W^2).T + 1e-8)               (B, COUT) host
  out[b] = d[b,:]/16 * blur(convT2x(s[b,:] * x[b], W)) + bias
- Modulation folds into x (per-input-channel scale), demodulation into the
  PSUM eviction (per-output-channel scale) -> weights stay sample-independent.
- Stride-2 transposed conv = 4 parity classes of <=2x2-tap convs on the 32x32
  input (subpixel decomposition; 9 effective taps instead of 36 dilated).
- Blur [1,3,3,1]^2/16 = three [1,1] passes per dim. Conv output is kept
  column-parity-split: planes E/O of the zero-padded 67-col grid, stored as
  FLAT [67*34] bf16 rows so every DVE op is one contiguous run (2x mode, no
  per-row bubbles). Even-phase W adds and all H adds are 4B-aligned (DVE 2x);
  the odd-phase adds (inherent +1-element offset) run on Pool (s1O, s2O) and
  DVE@1x (zzO). Pad columns swallow the wrap-around garbage; the final
  col-interleave + bias + fp32-convert on ACT reads only the valid columns.

Sharding: data-parallel over batch, 2 samples per core, 8 cores, no
collectives. Per-core engine budget: PE ~145us (832 matmuls, K=128, M=128,
N=330..512 — MAC-minimal for this decomposition), DVE ~108us, Pool ~80us,
ACT ~85us -> PE-bound.
"""

import copy
import os
from contextlib import ExitStack

import numpy as np
import ml_dtypes

import concourse.bass as bass
import concourse.tile as tile
from concourse import mybir
from concourse.bass_utils import run_bass_kernel_spmd

B, CIN, COUT, LAT, H, W_SP, KK = 16, 512, 512, 512, 32, 32, 3
NCORES = 8
BPC = B // NCORES  # samples per core
P = 128
NCI = CIN // P
NCO = COUT // P
BF16 = mybir.dt.bfloat16
F32 = mybir.dt.float32
PW = 34          # plane width (67-col padded grid split by col parity)
PL = 67 * PW     # plane flat length (2278)


_ENG_PREFIX = {
    "PE": "PE_", "DVE": "DVE_", "Activation": "Activation_",
    "Pool": "Pool_", "SP": "SP_",
}


def _fix_waits(nc: bass.Bass) -> None:
    """Walrus codegen accepts only one sem-wait per compute instruction;
    Tile emits up to 4.

    1) Drop same-engine self-waits: every engine executes its stream
       serially in order (PE matmul completion is pc-monotone; DVE/ACT
       have a hardware output-drain between ops), so a wait on the
       engine's own completion semaphore is redundant.
    2) Split any remaining multi-wait onto same-engine NoOp instructions
       inserted just before the instruction.
    """
    for f in nc.m.functions:
        for bb in f.blocks:
            out = []
            for inst in bb.instructions:
                si = inst.sync_info
                if si is None or len(si.on_wait) <= 1:
                    out.append(inst)
                    continue
                eng = str(inst.engine).split(".")[-1]
                pfx = _ENG_PREFIX.get(eng)
                waits = list(si.on_wait)
                keep = [
                    w for w in waits
                    if not (pfx and (w.ant_name or "").startswith(pfx))
                ]
                for w in keep[:-1]:
                    nop = mybir.InstNoOp(name=nc.get_next_instruction_name())
                    nop.engine = inst.engine
                    nop.sync_info = mybir.SyncInfo(on_wait=[w], on_update=[])
                    out.append(nop)
                keep = keep[-1:]
                inst.sync_info = mybir.SyncInfo(
                    on_wait=keep, on_update=list(si.on_update)
                )
                out.append(inst)
            bb.instructions = out


def build_program() -> bass.Bass:
    nc = bass.Bass()
    xp_d = nc.declare_dram_parameter("xp", [BPC, NCI, P, 34 * 34], BF16, isOutput=False)
    wt_d = nc.declare_dram_parameter("wt", [NCI, P, 9 * COUT], BF16, isOutput=False)
    dsc_d = nc.declare_dram_parameter("dsc", [P, BPC * NCO], F32, isOutput=False)
    bsc_d = nc.declare_dram_parameter("bsc", [P, NCO], F32, isOutput=False)
    out_d = nc.declare_dram_parameter("out", [BPC, NCO, P, 64 * 64], F32, isOutput=True)

    with ExitStack() as ctx:
        tc = ctx.enter_context(tile.TileContext(nc))
        consts = ctx.enter_context(tc.tile_pool(name="consts", bufs=1))
        xpool = ctx.enter_context(tc.tile_pool(name="xpool", bufs=1))
        psum = ctx.enter_context(tc.tile_pool(name="psum", bufs=8, space="PSUM"))
        spool = ctx.enter_context(tc.tile_pool(name="spool", bufs=2))
        spool1 = ctx.enter_context(tc.tile_pool(name="spool1", bufs=1))
        opool = ctx.enter_context(tc.tile_pool(name="opool", bufs=2))

        # DMA issue order matters: round 0 (s=0, oc=0, c-outer matmuls) can
        # start after wt chunk 0 + x(0,0) land (~1.4 MB), not the full 7 MB.
        w_sb = consts.tile([P, NCI, 9 * COUT], BF16, tag="wsb")
        d_sb = consts.tile([P, BPC * NCO], F32, tag="dsb")
        b_sb = consts.tile([P, NCO], F32, tag="bsb")
        x_tiles = {}

        def load_x(s, c):
            t = xpool.tile([P, 34, 34], BF16, tag=f"x{s}{c}", name=f"x{s}{c}")
            nc.sync.dma_start(
                out=t[:], in_=xp_d[s, c].rearrange("p (a b) -> p a b", b=34)
            )
            x_tiles[(s, c)] = t

        nc.sync.dma_start(out=w_sb[:, 0, :], in_=wt_d[0])
        load_x(0, 0)
        for c in range(1, NCI):
            nc.sync.dma_start(out=w_sb[:, c, :], in_=wt_d[c])
            load_x(0, c)
        nc.sync.dma_start(out=d_sb[:], in_=dsc_d[:])
        nc.sync.dma_start(out=b_sb[:], in_=bsc_d[:])
        for c in range(NCI):
            load_x(1, c)

        # Engine warm-up ops that absorb DMA-completion waits, so downstream
        # compute instructions stay within the 2-sem-wait ISA limit.
        warm_a = consts.tile([P, 1], F32, tag="warm_a")
        nc.scalar.copy(warm_a[:], d_sb[:, 0:1])
        warm_v = consts.tile([P, 1], F32, tag="warm_v")
        nc.vector.tensor_copy(warm_v[:], b_sb[:, 0:1])

        # Persistent column-parity planes of the zero-padded 67x67 grid,
        # stored flat ([67*34] + one pad row so shifted reads stay in
        # bounds). yE col m <-> padded col 2m ; yO col m <-> padded col
        # 2m+1 (col 33 = pad). Zeroed once; borders/pads stay zero,
        # interiors are fully overwritten by every eviction round.
        plane_sets = []
        for i in range(2):
            ye = consts.tile([P, PL + PW], BF16, tag=f"ye{i}")
            yo = consts.tile([P, PL + PW], BF16, tag=f"yo{i}")
            for t in (ye, yo):
                nc.scalar.memzero(t[:])
            plane_sets.append((ye, yo))

        for s in range(BPC):
            for oc in range(NCO):
                rnd = s * NCO + oc
                if rnd == 4:
                    # absorb the x(1,*) DMA sems before s=1 rounds
                    for c in range(NCI):
                        pwm = psum.tile([P, 512], F32, tag="ps", name=f"pswm{c}")
                        nc.tensor.matmul(
                            pwm[:, :16], w_sb[:, c, 0:P],
                            x_tiles[(1, c)][:, 0, 0:16],
                            start=True, stop=True,
                        )
                yE, yO = plane_sets[rnd % 2]
                for eh, ec in ((0, 0), (0, 1), (1, 0), (1, 1)):
                    rtaps = [(0, 0), (2, 1)] if eh == 0 else [(1, 1)]
                    ctaps = [(0, 0), (2, 1)] if ec == 0 else [(1, 1)]
                    ncols = 33 if ec == 0 else 32
                    if eh == 0:
                        rchunks = [(0, 11), (11, 11), (22, 11)]
                    elif ec == 0:
                        rchunks = [(0, 11), (11, 11), (22, 10)]
                    else:
                        rchunks = [(0, 16), (16, 16)]
                    taps = [(kh, kw, ra, cb) for (kh, ra) in rtaps for (kw, cb) in ctaps]
                    ptiles = [
                        psum.tile([P, 512], F32, tag="ps", name=f"ps{s}{oc}{eh}{ec}{fc}")
                        for fc in range(len(rchunks))
                    ]
                    nmm = len(taps) * NCI
                    i = 0
                    for c in range(NCI):          # c-outer: chunk-0 DMAs gate less
                        for kh, kw, ra, cb in taps:
                            toff = (kh * 3 + kw) * COUT + oc * P
                            lhsT = w_sb[:, c, toff : toff + P]
                            for fc, (u0, nr) in enumerate(rchunks):
                                rhs = x_tiles[(s, c)][:, u0 + ra : u0 + ra + nr,
                                                      cb : cb + ncols]
                                nc.tensor.matmul(
                                    ptiles[fc][:, : nr * ncols], lhsT, rhs,
                                    start=(i == 0), stop=(i == nmm - 1),
                                )
                            i += 1
                    # evict into the parity plane: padded row 1+eh+2u,
                    # padded col 1+ec+2v -> ec=0: yO col v ; ec=1: yE col v+1
                    plane = yO if ec == 0 else yE
                    col0 = 0 if ec == 0 else 1
                    pv = plane[:, 0:PL].rearrange("p (r c) -> p r c", c=PW)
                    for fc, (u0, nr) in enumerate(rchunks):
                        src = ptiles[fc][:, : nr * ncols].rearrange(
                            "p (r c) -> p r c", c=ncols
                        )
                        rsl = slice(1 + eh + 2 * u0, 1 + eh + 2 * (u0 + nr), 2)
                        nc.scalar.activation(
                            pv[:, rsl, col0 : col0 + ncols], src,
                            mybir.ActivationFunctionType.Copy,
                            bias=0.0,
                            scale=d_sb[:, rnd : rnd + 1],
                        )

                # --- W blur: three [1,1] passes per output col parity, all
                # flat 4B-aligned 2x TT adds:
                #   s1E[m]=y[2m]+y[2m+1]      s1O[m]=y[2m+1]+y[2m+2]
                #   s2*=s1 pair sums          zz*=s2 pair sums ([1,3,3,1])
                # The one-element-shifted operands (yEs=yE<<1, s1Es=s1E<<1,
                # s2Es=s2E<<1, s1Os=s1O<<1) are SBUF->SBUF DMA copies on the
                # otherwise-idle DMA rings — zero compute-engine cost.
                s1E = spool.tile([P, PL], BF16, tag="s1E", name=f"s1E{rnd}")
                s1O = spool.tile([P, PL], BF16, tag="s1O", name=f"s1O{rnd}")
                s2E = spool.tile([P, PL], BF16, tag="s2E", name=f"s2E{rnd}")
                s2O = spool.tile([P, PL], BF16, tag="s2O", name=f"s2O{rnd}")
                zzE = spool.tile([P, PL], BF16, tag="zzE", name=f"zzE{rnd}")
                zzO = spool.tile([P, PL], BF16, tag="zzO", name=f"zzO{rnd}")
                yEs = spool1.tile([P, PL], BF16, tag="yEs", name=f"yEs{rnd}")
                s1Es = spool1.tile([P, PL], BF16, tag="s1Es", name=f"s1Es{rnd}")
                s2Es = spool1.tile([P, PL], BF16, tag="s2Es", name=f"s2Es{rnd}")
                nc.sync.dma_start(out=yEs[:], in_=yE[:, 1 : PL + 1])
                nc.vector.tensor_add(s1E[:], yE[:, 0:PL], yO[:, 0:PL])
                nc.vector.tensor_add(s1O[:], yO[:, 0:PL], yEs[:])
                nc.sync.dma_start(out=s1Es[:, 0 : PL - 1], in_=s1E[:, 1:PL])
                nc.vector.tensor_add(s2E[:], s1E[:], s1O[:])
                nc.vector.tensor_add(s2O[:], s1O[:], s1Es[:])
                nc.sync.dma_start(out=s2Es[:, 0 : PL - 1], in_=s2E[:, 1:PL])
                nc.vector.tensor_add(zzE[:], s2E[:], s2O[:])
                nc.vector.tensor_add(zzO[:], s2O[:], s2Es[:])

                # --- H blur per plane: three flat row-shifted passes
                # (aligned, 2x). Scratch reuses the W tags (2nd ring slot).
                of = opool.tile([P, 64, 64], F32, tag="out", name=f"of{rnd}")
                for pw_, zp, t1, t2, t3 in (
                    (0, zzE, "s1E", "s2E", "zzE"),
                    (1, zzO, "s1O", "s2O", "zzO"),
                ):
                    c1 = spool.tile([P, PL], BF16, tag=t1, name=f"c1_{rnd}{pw_}")
                    nc.vector.tensor_add(
                        c1[:, 0 : 66 * PW], zp[:, 0 : 66 * PW], zp[:, PW : PL])
                    c2 = spool.tile([P, PL], BF16, tag=t2, name=f"c2_{rnd}{pw_}")
                    nc.vector.tensor_add(
                        c2[:, 0 : 65 * PW], c1[:, 0 : 65 * PW], c1[:, PW : 66 * PW])
                    c3 = spool.tile([P, PL], BF16, tag=t3, name=f"c3_{rnd}{pw_}")
                    nc.vector.tensor_add(
                        c3[:, 0 : 64 * PW], c2[:, 0 : 64 * PW], c2[:, PW : 65 * PW])
                    # col-interleave + bias + fp32 convert on ACT, split in
                    # row halves so the out-DMA can start early.
                    c3v = c3[:, 0 : 64 * PW].rearrange("p (r c) -> p r c", c=PW)
                    for rh in (0, 1):
                        nc.scalar.activation(
                            of[:, 32 * rh : 32 * (rh + 1), pw_ : 64 : 2],
                            c3v[:, 32 * rh : 32 * (rh + 1), 0:32],
                            mybir.ActivationFunctionType.Identity,
                            bias=b_sb[:, oc : oc + 1], scale=1.0,
                        )
                for rh in (0, 1):
                    nc.sync.dma_start(
                        out=out_d[s, oc, :, 2048 * rh : 2048 * (rh + 1)],
                        in_=of[:, 32 * rh : 32 * (rh + 1), :].rearrange(
                            "p a b -> p (a b)"),
                    )
    _fix_waits(nc)
    return nc


def make_in_maps(x, w, weight, bias, affine_w, affine_b):
    x = np.asarray(x, np.float32)
    w = np.asarray(w, np.float32)
    weight = np.asarray(weight, np.float32)
    bias = np.asarray(bias, np.float32)
    affine_w = np.asarray(affine_w, np.float32)
    affine_b = np.asarray(affine_b, np.float32)

    s = w @ affine_w.T + affine_b + 1.0  # (B, CIN)
    wsq = (weight.astype(np.float64) ** 2).sum(axis=(2, 3))  # (COUT, CIN)
    d = 1.0 / np.sqrt((s.astype(np.float64) ** 2) @ wsq.T + 1e-8)  # (B, COUT)
    d16 = (d / 16.0).astype(np.float32)

    xp = np.zeros((B, CIN, 34, 34), np.float32)
    xp[:, :, 1:33, 1:33] = x * s[:, :, None, None]
    xp_bf = xp.astype(ml_dtypes.bfloat16).reshape(B, NCI, P, 34 * 34)

    wf = weight[:, :, ::-1, ::-1]  # spatial flip
    wt = np.ascontiguousarray(
        wf.transpose(1, 2, 3, 0).reshape(NCI, P, 9 * COUT)
    ).astype(ml_dtypes.bfloat16)

    bsc = np.ascontiguousarray(bias.reshape(COUT).reshape(NCO, P).T).astype(np.float32)

    in_maps = []
    for core in range(NCORES):
        sl = slice(core * BPC, (core + 1) * BPC)
        dcore = d16[sl].reshape(BPC, NCO, P)
        dsc = np.ascontiguousarray(dcore.transpose(2, 0, 1).reshape(P, BPC * NCO))
        in_maps.append(
            {
                "xp": np.ascontiguousarray(xp_bf[sl]),
                "wt": wt,
                "dsc": dsc,
                "bsc": bsc,
            }
        )
    return in_maps


LAST_RESULTS = None  # BassKernelResults of the most recent run (for test harness)


def kernel(x, w, weight, bias, affine_w, affine_b):
    global LAST_RESULTS
    in_maps = make_in_maps(x, w, weight, bias, affine_w, affine_b)
    nc = build_program()
    res = run_bass_kernel_spmd(nc, in_maps, list(range(NCORES)))
    LAST_RESULTS = res
    outs = [r["out"].reshape(BPC, COUT, 64, 64) for r in res.results]
    return np.ascontiguousarray(np.concatenate(outs, axis=0), dtype=np.float32)

